# revision 5
# baseline (speedup 1.0000x reference)
"""BlockWiseEmbedding gather kernel for 8 Trainium2 NeuronCores.

Data-parallel over tokens, table replicated, bf16 end-to-end.
out[b, t] = tables_concat[offsets[block_assignment[src[b,t]]] + local_assignment[src[b,t]]]

The host computes the flat row per token, downcasts the table to bf16
(rel-err gate is 2e-2; bf16 rounding is ~4e-3) — halving both the
gather-read and store-write HBM traffic vs f32 — and sorts each core's
8192 tokens by row id (host un-permutes rows when assembling the
output), so the gather walks the table in ascending row order for HBM
page locality. Each core gathers 8192 rows of 1KB via indirect DMA
(one descriptor per row, 128 per instruction) and streams them out,
pipelined via Tile.
"""
import functools

import ml_dtypes
import numpy as np

import concourse.bacc as bacc
import concourse.bass as bass
import concourse.mybir as mybir
import concourse.tile as tile
from concourse.bass_utils import run_bass_kernel_spmd

# Problem shape (hardcoded per the harness contract).
BATCH, SEQ = 32, 2048
N_TOK = BATCH * SEQ
VOCAB = 100000
DIM = 512
N_CORES = 8
P = 128
TOK_PER_CORE = N_TOK // N_CORES            # 8192
COLS = TOK_PER_CORE // P                   # 64 tokens per partition
STORE_K = 2                                # gathered columns per output store


@functools.lru_cache(maxsize=1)
def _build():
    nc = bacc.Bacc("TRN2", target_bir_lowering=False, debug=False)
    idx_h = nc.dram_tensor("idx", [P, COLS], mybir.dt.int32, kind="ExternalInput")
    tab_h = nc.dram_tensor(
        "table", [VOCAB, DIM], mybir.dt.bfloat16, kind="ExternalInput"
    )
    out_h = nc.dram_tensor(
        "out", [TOK_PER_CORE, DIM], mybir.dt.bfloat16, kind="ExternalOutput"
    )
    # Token t = p*COLS + c lives at SBUF partition p, column c.
    out_v = out_h.ap().rearrange("(p c) d -> p c d", p=P)

    n_batches = COLS // STORE_K
    with tile.TileContext(nc) as tc:
        with (
            tc.tile_pool(name="g", bufs=n_batches) as gpool,
            tc.tile_pool(name="ix", bufs=1) as ixpool,
        ):
            idx_tile = ixpool.tile([P, COLS], mybir.dt.int32)
            nc.sync.dma_start(out=idx_tile[:], in_=idx_h[:])
            # HW indirect DMA moves one 1KB row per partition per
            # instruction; batch STORE_K of them per output store.
            for bi in range(n_batches):
                g = gpool.tile([P, STORE_K * DIM], mybir.dt.bfloat16)
                for j in range(STORE_K):
                    ci = bi * STORE_K + j
                    nc.gpsimd.indirect_dma_start(
                        out=g[:, j * DIM:(j + 1) * DIM],
                        out_offset=None,
                        in_=tab_h[:],
                        in_offset=bass.IndirectOffsetOnAxis(
                            ap=idx_tile[:, ci:ci + 1], axis=0
                        ),
                    )
                # Stores alternate across the two HWDGE rings (sync/scalar).
                store_eng = nc.sync if bi % 2 == 0 else nc.scalar
                store_eng.dma_start(
                    out=out_v[:, bi * STORE_K:(bi + 1) * STORE_K, :], in_=g[:]
                )

    nc.compile()
    return nc


def _prepare(src, block_assignment, local_assignment, tables):
    """Host-side routing: flat row per token; per-core sort by row id."""
    src = np.asarray(src).astype(np.int64)
    blk = np.asarray(block_assignment).astype(np.int64)
    loc = np.asarray(local_assignment).astype(np.int64)
    sizes = np.array([t.shape[0] for t in tables], dtype=np.int64)
    offsets = np.concatenate([np.zeros(1, np.int64), np.cumsum(sizes)[:-1]])
    rows = (offsets[blk[src]] + loc[src]).reshape(-1)   # [N_TOK]
    big = np.ascontiguousarray(
        np.concatenate(
            [np.asarray(t, dtype=np.float32) for t in tables], axis=0
        ).astype(ml_dtypes.bfloat16)
    )

    in_maps, g_idx = [], np.empty(N_TOK, np.int64)
    pos2row = np.arange(TOK_PER_CORE)
    for c in range(N_CORES):
        sl = slice(c * TOK_PER_CORE, (c + 1) * TOK_PER_CORE)
        order = np.argsort(rows[sl], kind="stable")     # ascending table rows
        idx_c = rows[sl][order].astype(np.int32).reshape(P, COLS)
        in_maps.append({"idx": np.ascontiguousarray(idx_c), "table": big})
        # sorted-list position q -> SBUF (p=q//COLS, c=q%COLS) -> DRAM row q;
        # original token (sl.start + order[q]) gets that row.
        g_idx[c * TOK_PER_CORE + order] = c * TOK_PER_CORE + pos2row
    return in_maps, g_idx


def run(inputs, trace=False):
    """Shard, execute on 8 cores, return (full_output, BassKernelResults)."""
    in_maps, g_idx = _prepare(
        inputs["src"],
        inputs["block_assignment"],
        inputs["local_assignment"],
        [inputs["table0"], inputs["table1"], inputs["table2"], inputs["table3"]],
    )
    nc = _build()
    # Device execution is occasionally flaky on a fresh NEFF
    # (NRT_EXEC_UNIT_UNRECOVERABLE); an identical retry succeeds.
    last_err = None
    for _ in range(3):
        try:
            res = run_bass_kernel_spmd(
                nc, in_maps, core_ids=list(range(N_CORES)), trace=trace
            )
            break
        except Exception as e:  # noqa: BLE001
            last_err = e
    else:
        raise last_err
    hw = np.concatenate(
        [np.asarray(r["out"]).astype(np.float32) for r in res.results], axis=0
    )
    return hw[g_idx].reshape(BATCH, SEQ, DIM), res


def kernel(**inputs) -> np.ndarray:
    out, _ = run(inputs)
    return out
